# Initial kernel scaffold
#
"""Trainium2 Bass kernel for nn_Covariance.

Math: for Xs [B,T,F,2,M], the reference forms per-(b,t,f) upper-triangular
complex covariance entries and then replaces them with their time-mean
(broadcast back over T). Writing x_tf = (re||im) in R^16, every needed
quantity is an entry of the time-summed Gram matrix C_f = sum_t x_tf x_tf^T:

    re_part(i,j) = C[i, j]   + C[8+i, 8+j]
    im_part(i,j) = C[i, 8+j] - C[j, 8+i]

so the device kernel only computes C_f (16x16 per frequency) via PE matmuls
with the T-contraction on the partition axis (PSUM accumulates the 4 chunks
of T=512).  The tiny recombination + /T + time-broadcast happen on host.

Sharding: batch-parallel, one batch element per NeuronCore (B == 8 cores).
Per core: read 16.8 MB, write 513*16*16 floats (525 KB).
"""

import numpy as np

_B, _T, _F, _M = 8, 512, 513, 8
_CH = 2 * _M            # 16 packed re/im channels
_KC = _T // 128         # 4 chunks of the time axis (PSUM-accumulated)
_NCORES = 8
_BANK = 512             # fp32 columns per PSUM bank
_FPB = _BANK // _CH     # 32 frequencies per PSUM bank
_NBANKS = (_F + _FPB - 1) // _FPB   # 17 (last bank holds 1 frequency)

_nc_cache = None


def _build_nc():
    import concourse.mybir as mybir
    from concourse import bacc, tile

    f32 = mybir.dt.float32
    nc = bacc.Bacc(None, target_bir_lowering=False)
    x = nc.declare_dram_parameter("x", [_T, _F * _CH], f32, isOutput=False)
    gram = nc.declare_dram_parameter("gram", [_CH, _F * _CH], f32, isOutput=True)

    with tile.TileContext(nc) as tc:
        with (
            tc.tile_pool(name="xin", bufs=3 * _KC) as xpool,
            tc.tile_pool(name="ps", bufs=6, space="PSUM") as ppool,
            tc.tile_pool(name="out", bufs=4) as opool,
        ):
            for bank in range(_NBANKS):
                f0 = bank * _FPB
                nf = min(_FPB, _F - f0)
                w = nf * _CH
                xts = []
                for kc in range(_KC):
                    xt = xpool.tile([128, _BANK], f32, tag="x")
                    nc.sync.dma_start(
                        xt[:, :w],
                        x[kc * 128:(kc + 1) * 128, f0 * _CH:f0 * _CH + w],
                    )
                    xts.append(xt)
                pt = ppool.tile([_CH, _BANK], f32, tag="ps")
                for fl in range(nf):
                    sl = slice(fl * _CH, (fl + 1) * _CH)
                    for kc in range(_KC):
                        nc.tensor.matmul(
                            pt[:, sl],
                            xts[kc][:, sl],
                            xts[kc][:, sl],
                            start=(kc == 0),
                            stop=(kc == _KC - 1),
                        )
                ot = opool.tile([_CH, _BANK], f32, tag="o")
                nc.vector.tensor_copy(ot[:, :w], pt[:, :w])
                nc.sync.dma_start(
                    gram[:, f0 * _CH:f0 * _CH + w], ot[:, :w]
                )

    nc.compile()
    return nc


def kernel(Xs):
    global _nc_cache
    from concourse.bass_utils import run_bass_kernel_spmd

    Xs = np.asarray(Xs, dtype=np.float32)
    assert Xs.shape == (_B, _T, _F, 2, _M)
    if _nc_cache is None:
        _nc_cache = _build_nc()

    xs2 = Xs.reshape(_B, _T, _F * _CH)
    in_maps = [{"x": xs2[b]} for b in range(_B)]
    res = run_bass_kernel_spmd(_nc_cache, in_maps, list(range(_NCORES))).results

    C = np.stack([r["gram"] for r in res])              # [B, 16, F*16]
    C = C.reshape(_B, _CH, _F, _CH).transpose(0, 2, 1, 3)  # [B, F, 16, 16]
    iu0, iu1 = np.triu_indices(_M)
    re = C[:, :, iu0, iu1] + C[:, :, _M + iu0, _M + iu1]
    im = C[:, :, iu0, _M + iu1] - C[:, :, iu1, _M + iu0]
    mean = np.stack([re, im], axis=2) * np.float32(1.0 / _T)  # [B, F, 2, 36]
    mean = np.ascontiguousarray(mean, dtype=np.float32)
    npairs = _M * (_M + 1) // 2
    return np.broadcast_to(
        mean[:, None], (_B, _T, _F, 2, npairs)
    )


# revision 3
# speedup vs baseline: 1.6421x; 1.6421x over previous
"""Trainium2 Bass kernel for nn_Covariance.

Math: for Xs [B,T,F,2,M], the reference forms per-(b,t,f) upper-triangular
complex covariance entries and then replaces them with their time-mean
(broadcast back over T). Writing x_tf = (re||im) in R^16, every needed
quantity is an entry of the time-summed Gram matrix C_f = sum_t x_tf x_tf^T:

    re_part(i,j) = C[i, j]   + C[8+i, 8+j]
    im_part(i,j) = C[i, 8+j] - C[j, 8+i]

so the device kernel only computes C_f (16x16 per frequency) via PE matmuls
with the T-contraction on the partition axis (PSUM accumulates the 4 chunks
of T=512).  The tiny recombination + /T + time-broadcast happen on host.

Sharding: batch-parallel, one batch element per NeuronCore (B == 8 cores).
Per core: read 16.8 MB, write 513*16*16 floats (525 KB).
"""

import numpy as np

_B, _T, _F, _M = 8, 512, 513, 8
_CH = 2 * _M            # 16 packed re/im channels
_KC = _T // 128         # 4 chunks of the time axis (PSUM-accumulated)
_NCORES = 8
_BANK = 512             # fp32 columns per PSUM bank
_FPB = _BANK // _CH     # 32 frequencies per PSUM bank
_NBANKS = (_F + _FPB - 1) // _FPB   # 17 (last bank holds 1 frequency)

_nc_cache = None


def _build_nc(reps=1):
    import concourse.mybir as mybir
    from concourse import bacc, tile

    f32 = mybir.dt.float32
    nc = bacc.Bacc(None, target_bir_lowering=False)
    x = nc.declare_dram_parameter("x", [_T, _F * _CH], f32, isOutput=False)
    gram = nc.declare_dram_parameter("gram", [_CH, _F * _CH], f32, isOutput=True)

    with tile.TileContext(nc) as tc:
        with (
            tc.tile_pool(name="xin", bufs=3 * _KC) as xpool,
            tc.tile_pool(name="ps", bufs=6, space="PSUM") as ppool,
            tc.tile_pool(name="out", bufs=4) as opool,
        ):
            for bank in range(_NBANKS * reps):
                bank = bank % _NBANKS
                f0 = bank * _FPB
                nf = min(_FPB, _F - f0)
                w = nf * _CH
                xts = []
                for kc in range(_KC):
                    xt = xpool.tile([128, _BANK], f32, tag="x")
                    nc.sync.dma_start(
                        xt[:, :w],
                        x[kc * 128:(kc + 1) * 128, f0 * _CH:f0 * _CH + w],
                    )
                    xts.append(xt)
                pt = ppool.tile([_CH, _BANK], f32, tag="ps")
                for fl in range(nf):
                    sl = slice(fl * _CH, (fl + 1) * _CH)
                    for kc in range(_KC):
                        nc.tensor.matmul(
                            pt[:, sl],
                            xts[kc][:, sl],
                            xts[kc][:, sl],
                            start=(kc == 0),
                            stop=(kc == _KC - 1),
                        )
                ot = opool.tile([_CH, _BANK], f32, tag="o")
                nc.vector.tensor_copy(ot[:, :w], pt[:, :w])
                nc.sync.dma_start(
                    gram[:, f0 * _CH:f0 * _CH + w], ot[:, :w]
                )

    nc.compile()
    return nc


def _build_nc_reps(reps):
    return _build_nc(reps)


def kernel(Xs):
    global _nc_cache
    from concourse.bass_utils import run_bass_kernel_spmd

    Xs = np.asarray(Xs, dtype=np.float32)
    assert Xs.shape == (_B, _T, _F, 2, _M)
    if _nc_cache is None:
        _nc_cache = _build_nc()

    xs2 = Xs.reshape(_B, _T, _F * _CH)
    in_maps = [{"x": xs2[b]} for b in range(_B)]
    res = run_bass_kernel_spmd(_nc_cache, in_maps, list(range(_NCORES))).results

    C = np.stack([r["gram"] for r in res])              # [B, 16, F*16]
    C = C.reshape(_B, _CH, _F, _CH).transpose(0, 2, 1, 3)  # [B, F, 16, 16]
    iu0, iu1 = np.triu_indices(_M)
    re = C[:, :, iu0, iu1] + C[:, :, _M + iu0, _M + iu1]
    im = C[:, :, iu0, _M + iu1] - C[:, :, iu1, _M + iu0]
    mean = np.stack([re, im], axis=2) * np.float32(1.0 / _T)  # [B, F, 2, 36]
    mean = np.ascontiguousarray(mean, dtype=np.float32)
    npairs = _M * (_M + 1) // 2
    return np.broadcast_to(
        mean[:, None], (_B, _T, _F, 2, npairs)
    )
